# revision 15
# baseline (speedup 1.0000x reference)
"""Trainium2 Bass kernel for EfmLSTM (signature-gated LSTM), 8-core data-parallel.

Strategy
--------
Data-parallel over batch: B=64 -> 8 cores x B_loc=8. Everything on-chip uses a
"units-on-partition" transposed layout so the sequential scan needs no
transposes at all: h^T, c^T and gate tensors are [128 partitions, k*8+b]
where unit = 128*k+p.

Per-step critical chain (the whole point of this design — T=1024 sequential
steps mean total time = T x chain latency):

  1. PE: one identity matmul seeds the gate PSUM bank [128,128] with the
     precomputed per-step block [x_c | x_i/2 | x_o/2 | zf/2] (start=True), then
     48 bf16 matmuls (12 gate-chunks x 4 h-chunks, N=8) accumulate R'^T h2.
  2. ACT: ONE Tanh over the whole bank. Sigmoid gates use
     sigma(z) = (tanh(z/2)+1)/2 with i/o/f pre-halved on the host, so a single
     activation function covers c~, i, o AND f.
  3. DVE: P=(tf+1)*C, Q=(ti+1)*tc, C=0.5P+Q   (C == 2c; exact, not approx)
  4. ACT: u = tanh(0.5*C)
  5. DVE: h2 = (to+1)*u  (h2 == 2h; R' = R/2 folded on host, output halved
     on host)

This removes 2 of the 4 per-step activations, the 3 per-step DVE x-copies of
the old design, and the f-gate's separate sigmoid eviction. x/f precompute
(full-width N=512 matmuls) is interleaved into the scan's PE idle windows;
its PSUM->SBUF evictions run on DVE right after h2 so they never block the
critical ACT ops.

The T loop is a hardware For_i over chunk PAIRS (ping-pong SBUF slots inside
the body). Host side pre-permutes weights into gate order [c~, i, o] (+f),
applies the tanh-domain scalings, and pre-transposes / pre-casts inputs to
bf16, so the device never transposes anything.
"""

import numpy as np
import ml_dtypes

# Problem shapes (hardcoded per harness contract)
B, T, F = 64, 1024, 256
U = 512
SIG = 31
NCORES = 8
BL = B // NCORES  # 8 batch per core

T_CHUNK = 128
KC = U // 128        # 4 k-chunks of h/units
MC = (3 * U) // 128  # 12 gate-column chunks (c~, i, o)
GC = MC + KC         # 16 bank chunks (c~, i, o, f)
FC = F // 128        # 2 k-chunks of input features
W = KC * BL          # 32 (one gate's width in the transposed layout)
BANK = GC * BL       # 128 (gate PSUM bank width)

_cache = {}

# instruction-name -> role map, populated at build time (used by the local
# timeline-sim analysis scripts; harmless in production)
ROLES = {}


def _role(inst, role):
    try:
        ROLES[inst.ins.name] = role
    except Exception:
        pass
    return inst


DROP_SAME_ENGINE_WAITS = True


def _split_excess_waits(nc, limit=1):
    """Post-pass over sync_info:

    1. Drop semaphore waits on the instruction's OWN engine's sem - engines
       execute their queue in order, so program order already serializes
       same-engine producer/consumer pairs; the sem round-trip (~90ns) is
       pure critical-path cost. (DMA/event sems and cross-engine sems kept.)
    2. This walrus build rejects >1 sync-wait command per instruction
       ('Too many sync wait commands', CoreV2/V3 setupSyncWait). Hoist excess
       waits onto same-engine NoOp instructions inserted just before the
       offending instruction. Keep the LATEST-RESOLVING wait on the
       instruction itself (heuristic: prefer the Activation-sem wait - in the
       scan chain ACT is always the late producer), so the critical wait is
       not stuck behind an extra NoOp decode.
    """
    import concourse.mybir as mybir
    import bass_rust as _br

    own_droppable = {"PE", "Activation", "DVE", "Pool"}
    cnt = 0
    for f in nc.m.functions:
        for b in f.blocks:
            il = b.instructions
            new = []
            for inst in il:
                si = inst.sync_info
                waits = list(si.on_wait) if si and si.on_wait else []
                if not waits:
                    new.append(inst)
                    continue
                eng = str(inst.engine).split(".")[-1]
                if DROP_SAME_ENGINE_WAITS and eng in own_droppable:
                    kept = []
                    for w in waits:
                        nm = (w.ant_name or "") if w.sync_type == "semaphore" else ""
                        pref = nm.rsplit("_", 1)[0] if nm else ""
                        if pref == eng:
                            continue
                        kept.append(w)
                    waits = kept
                if len(waits) > limit:
                    act_i = None
                    for i_, w in enumerate(waits):
                        nm = (w.ant_name or "") if w.sync_type == "semaphore" else ""
                        if nm.startswith("Activation"):
                            act_i = i_
                    if act_i is not None:
                        waits.append(waits.pop(act_i))
                    for w in waits[:-limit]:
                        nop = mybir.InstNoOp(name=f"wsplit_{cnt}", ins=[], outs=[])
                        cnt += 1
                        nop.engine = inst.engine
                        nop.sync_info = _br.SyncInfo(on_wait=[w], on_update=[])
                        new.append(nop)
                    waits = waits[-limit:]
                si.on_wait = waits
                new.append(inst)
            il[:] = new
    return cnt


def _build_nc(compute_dt_name="bfloat16", t_eff=T):
    import concourse.bass as bass
    import concourse.mybir as mybir
    import concourse.tile as tile
    from contextlib import ExitStack

    fp32 = mybir.dt.float32
    cdt = getattr(mybir.dt, compute_dt_name)
    AF = mybir.ActivationFunctionType
    ALU = mybir.AluOpType
    ds = bass.ds

    assert t_eff % (2 * T_CHUNK) == 0
    nc = bass.Bass()

    # t dim padded by one body (2*T_CHUNK) so the loop's next-iteration
    # prefetch never reads out of bounds
    t_pad = t_eff + 2 * T_CHUNK
    x_in = nc.declare_dram_parameter("inputs_t", [F, BL, t_pad], cdt, isOutput=False)
    sig_in = nc.declare_dram_parameter("sig_t", [SIG, BL, t_pad], cdt, isOutput=False)
    wrec_in = nc.declare_dram_parameter("wrec", [128, KC * MC * 128], cdt, isOutput=False)
    win_in = nc.declare_dram_parameter("win", [128, FC * MC * 128], cdt, isOutput=False)
    wsig_in = nc.declare_dram_parameter("wsig", [SIG, U], cdt, isOutput=False)
    ident_in = nc.declare_dram_parameter("ident", [128, 128], cdt, isOutput=False)
    bias_g_in = nc.declare_dram_parameter("bias_g", [128, GC], fp32, isOutput=False)
    h_out = nc.declare_dram_parameter("h_out", [128, KC * BL], fp32, isOutput=True)

    NT = 512 // BL  # timesteps covered per 512-wide precompute matmul

    with ExitStack() as ctx:
        tc = ctx.enter_context(tile.TileContext(nc))

        const = ctx.enter_context(tc.tile_pool(name="const", bufs=1))
        state = ctx.enter_context(tc.tile_pool(name="state", bufs=1))
        data = ctx.enter_context(tc.tile_pool(name="data", bufs=1))
        work = ctx.enter_context(tc.tile_pool(name="work", bufs=3))
        psum_g = ctx.enter_context(tc.tile_pool(name="psum_g", bufs=2, space="PSUM"))
        psum_p = ctx.enter_context(tc.tile_pool(name="psum_p", bufs=2, space="PSUM"))

        wrec = const.tile([128, KC * MC * 128], cdt)
        nc.sync.dma_start(out=wrec[:], in_=wrec_in[:])
        win = const.tile([128, FC * MC * 128], cdt)
        nc.sync.dma_start(out=win[:], in_=win_in[:])
        wsig = const.tile([SIG, U], cdt)
        nc.sync.dma_start(out=wsig[:], in_=wsig_in[:])
        ident = const.tile([128, 128], cdt)
        nc.sync.dma_start(out=ident[:], in_=ident_in[:])
        bias_g = const.tile([128, GC], fp32)
        nc.sync.dma_start(out=bias_g[:], in_=bias_g_in[:])

        h_bf = state.tile([128, W], cdt)       # h2^T bf16, col = 8*k + b
        c_st = state.tile([128, W], fp32)      # C^T = 2c fp32
        nc.vector.memset(h_bf[:], 0.0)
        nc.vector.memset(c_st[:], 0.0)

        srcv = x_in.rearrange("(k p) b t -> p k b t", p=128)

        # per-half static tile sets (ping-pong inside the For_i body)
        halves = []
        for hf in range(2):
            in_sb = data.tile([128, FC * BL * T_CHUNK], cdt, name=f"in_sb{hf}")
            sig_sb = data.tile([SIG, BL * T_CHUNK], cdt, name=f"sig_sb{hf}")
            x_sb = data.tile([128, T_CHUNK * BANK], cdt, name=f"x_sb{hf}")
            halves.append((in_sb, sig_sb, x_sb))

        def emit_dmas(t0sc, hf):
            """Stage the half's inputs+signatures from DRAM."""
            in_sb, sig_sb, x_sb = halves[hf]
            in_sbv = in_sb.rearrange("p (k b t) -> p k b t", k=FC, b=BL)
            for k in range(FC):
                nc.sync.dma_start(out=in_sbv[:, k, :, :],
                                  in_=srcv[:, k, :, ds(t0sc, T_CHUNK)])
            nc.sync.dma_start(out=sig_sb.rearrange("p (b t) -> p b t", b=BL),
                              in_=sig_in[:, :, ds(t0sc, T_CHUNK)])

        def pre_groups(hf):
            """Generator: x/f precompute for the half, interleaved into the
            scan's idle windows. Each group yields TWICE — once after its
            matmuls, once after its eviction — so the eviction's input is
            ready a step before the eviction DVE op is emitted and its 658ns
            burst lands in the DVE idle window right after h2.
            Evicted block layout: x_sb col = t*BANK + j*BL + b with chunk
            order [c~ x4 | i x4 | o x4 | f x4]."""
            in_sb, sig_sb, x_sb = halves[hf]
            in_v = in_sb.rearrange("p (k b t) -> p k b t", k=FC, b=BL)
            sig_v = sig_sb.rearrange("p (b t) -> p b t", b=BL)
            x_v = x_sb.rearrange("p (t g b) -> p g b t", g=GC, b=BL)
            for j in range(GC):
                for th in range(T_CHUNK // NT):
                    ps = psum_p.tile([128, 512], fp32, tag="ps_pre", name="ps_pre")
                    if j < MC:
                        for k in range(FC):
                            _role(nc.tensor.matmul(
                                ps[:],
                                lhsT=win[:, (k * MC + j) * 128:(k * MC + j + 1) * 128],
                                rhs=in_v[:, k, :, ds(th * NT, NT)],
                                start=(k == 0), stop=(k == FC - 1),
                            ), "pre-mm")
                    else:
                        uu = j - MC
                        _role(nc.tensor.matmul(
                            ps[:],
                            lhsT=wsig[:, uu * 128:(uu + 1) * 128],
                            rhs=sig_v[:, :, ds(th * NT, NT)],
                            start=True, stop=True,
                        ), "pre-mm-f")
                    yield
                    ps_v = ps.rearrange("p (b t) -> p b t", b=BL)
                    _role(nc.scalar.activation(
                        x_v[:, j, :, ds(th * NT, NT)], ps_v[:, :, :],
                        AF.Identity, bias=bias_g[:, j:j + 1]), "pre-evict")
                    yield

        def scan(hf, pre_iter):
            in_sb, sig_sb, x_sb = halves[hf]
            to_t = u_t = None
            for tt in range(T_CHUNK):
                # full-bank tile so the two ping-pong slots never share a
                # physical PSUM bank (bank-granular accumulate tracking would
                # otherwise serialize the seed behind the other slot's reads)
                pg = psum_g.tile([128, 512], fp32, tag="pg", name="pg")
                # seed the bank with [x_c | x_i/2 | x_o/2 | zf/2] in one
                # identity matmul (start=True resets has_written); runs well
                # before h2 lands since it only needs x_sb + the bank free
                _role(nc.tensor.matmul(
                    pg[:, 0:BANK], lhsT=ident[:],
                    rhs=x_sb[:, tt * BANK:(tt + 1) * BANK],
                    start=True, stop=True, skip_group_check=True), "id-mm")
                # f has no recurrent term -> its tanh and P=(tf+1)*C run right
                # after the seed, entirely inside the h2-wait window
                tf_t = work.tile([128, W], fp32, tag="tf", name="tf")
                _role(nc.scalar.activation(tf_t[:], pg[:, 3 * W:4 * W], AF.Tanh),
                      "act-f")
                P = work.tile([128, W], fp32, tag="P", name="P")
                _role(nc.vector.scalar_tensor_tensor(
                    P[:], tf_t[:], 1.0, c_st[:], op0=ALU.add, op1=ALU.mult),
                      "dve-P")
                # 48 recurrent matmuls accumulate R'^T h2 on top; c~ and i
                # first so their tanh can start before the o block finishes
                for j in range(MC):
                    for k in range(KC):
                        _role(nc.tensor.matmul(
                            pg[:, j * BL:(j + 1) * BL],
                            lhsT=wrec[:, (k * MC + j) * 128:(k * MC + j + 1) * 128],
                            rhs=h_bf[:, k * BL:(k + 1) * BL],
                            start=False, stop=(k == KC - 1),
                            skip_group_check=True), f"mm-{j}-{k}")
                # critical tanh over [c~ | i] only
                G_t = work.tile([128, 2 * W], fp32, tag="G", name="G")
                _role(nc.scalar.activation(G_t[:], pg[:, 0:2 * W], AF.Tanh),
                      "act-G")
                # C = 0.5*(tf+1)*C + (ti+1)*tc   (== 2c, exact)
                Q = work.tile([128, W], fp32, tag="Q", name="Q")
                _role(nc.vector.scalar_tensor_tensor(
                    Q[:], G_t[:, W:2 * W], 1.0, G_t[:, 0:W],
                    op0=ALU.add, op1=ALU.mult), "dve-Q")
                _role(nc.vector.scalar_tensor_tensor(
                    c_st[:], P[:], 0.5, Q[:], op0=ALU.mult, op1=ALU.add),
                      "dve-C")
                # o's tanh fills the ACT gap while DVE runs the C chain
                to_t = work.tile([128, W], fp32, tag="to", name="to")
                _role(nc.scalar.activation(to_t[:], pg[:, 2 * W:3 * W], AF.Tanh),
                      "act-o")
                u_t = work.tile([128, W], fp32, tag="u", name="u")
                _role(nc.scalar.activation(u_t[:], c_st[:], AF.Tanh, scale=0.5),
                      "act-u")
                # h2 = (to+1)*u
                _role(nc.vector.scalar_tensor_tensor(
                    h_bf[:], to_t[:], 1.0, u_t[:], op0=ALU.add, op1=ALU.mult),
                      "dve-h2")
                # fill the PE/DVE h2-wait idle window with precompute work
                # (64 yields per half: matmul stage and evict stage alternate)
                if pre_iter is not None and tt % 2 == 1:
                    next(pre_iter, None)
            if pre_iter is not None:
                for _ in pre_iter:
                    pass
            return to_t, u_t

        # prologue: half0 of the first iteration, serial
        emit_dmas(0, 0)
        for _ in pre_groups(0):
            pass

        with tc.For_i(0, t_eff, 2 * T_CHUNK) as t0:
            emit_dmas(t0 + T_CHUNK, 1)           # this iteration's half1 data
            scan(0, pre_groups(1))               # scan half0, prep half1
            emit_dmas(t0 + 2 * T_CHUNK, 0)       # next iteration's half0 data
            to_l, u_l = scan(1, pre_groups(0))   # scan half1, prep next half0

        # final h2 in fp32 from the last step's stashed (static-slot) tiles
        h_f = state.tile([128, W], fp32)
        nc.vector.scalar_tensor_tensor(
            h_f[:], to_l[:], 1.0, u_l[:], op0=ALU.add, op1=ALU.mult)
        nc.sync.dma_start(out=h_out[:], in_=h_f[:])

    _split_excess_waits(nc)
    return nc


def _prep_host_inputs(inputs, signatures, forget_kernel, input_kernel,
                      recurrent_kernel, bias, cdt=ml_dtypes.bfloat16, t_factor=1):
    """Host-side shard + permute + scale + transpose + cast.

    tanh-domain scalings: sigma(z) = (tanh(z/2)+1)/2 for i/o/f -> their
    preactivation columns are halved; the device carries h2=2h and C=2c, so
    recurrent columns get an extra /2 (R' = R/2). Output h = h2/2 (host).
    """
    # gate order in reference: [i, c~, o]; ours: [c~, i, o] (+f block)
    perm = np.concatenate([np.arange(U, 2 * U), np.arange(0, U), np.arange(2 * U, 3 * U)])
    win_p = input_kernel[:, perm].copy()       # [F, 3U]
    wrec_p = recurrent_kernel[:, perm].copy()  # [U, 3U]
    b_i, b_f, b_c, b_o = np.split(bias, 4)

    win_p[:, U:3 * U] *= 0.5                   # i, o halved (tanh domain)
    wrec_p[:, 0:U] *= 0.5                      # c~: /2 (h2 domain)
    wrec_p[:, U:3 * U] *= 0.25                 # i, o: /2 (h2) * /2 (tanh)
    wsig = (forget_kernel * 0.5).astype(cdt)   # [SIG, U], f halved
    bias_vec = np.concatenate([b_c, 0.5 * b_i, 0.5 * b_o, 0.5 * b_f])  # [4U]
    bg = bias_vec.reshape(GC, 128).T.copy()    # [128, GC]

    # wrec blocks: [128, (k*MC + j)*128 + c] = wrec_p[128*k + p, 128*j + c]
    wr = wrec_p.reshape(KC, 128, MC, 128).transpose(1, 0, 2, 3).reshape(128, KC * MC * 128)
    wi = win_p.reshape(FC, 128, MC, 128).transpose(1, 0, 2, 3).reshape(128, FC * MC * 128)
    wr = wr.astype(cdt)
    wi = wi.astype(cdt)
    ident = np.eye(128, dtype=cdt)

    in_maps = []
    for c in range(NCORES):
        bsl = slice(c * BL, (c + 1) * BL)
        # [BL, T, F] -> [F, BL, T]
        x_t = np.ascontiguousarray(inputs[bsl].transpose(2, 0, 1)).astype(cdt)
        s_t = np.ascontiguousarray(signatures[bsl].transpose(2, 0, 1)).astype(cdt)
        if t_factor > 1:
            x_t = np.tile(x_t, (1, 1, t_factor))
            s_t = np.tile(s_t, (1, 1, t_factor))
        pad = 2 * T_CHUNK
        x_t = np.concatenate([x_t, np.zeros(x_t.shape[:2] + (pad,), x_t.dtype)], axis=2)
        s_t = np.concatenate([s_t, np.zeros(s_t.shape[:2] + (pad,), s_t.dtype)], axis=2)
        in_maps.append({
            "inputs_t": x_t, "sig_t": s_t, "wrec": wr, "win": wi,
            "wsig": wsig, "ident": ident,
            "bias_g": bg.astype(np.float32),
        })
    return in_maps


def kernel(inputs, signatures, forget_kernel, input_kernel, recurrent_kernel,
           bias, _trace=False):
    inputs = np.asarray(inputs, dtype=np.float32)
    signatures = np.asarray(signatures, dtype=np.float32)
    forget_kernel = np.asarray(forget_kernel, dtype=np.float32)
    input_kernel = np.asarray(input_kernel, dtype=np.float32)
    recurrent_kernel = np.asarray(recurrent_kernel, dtype=np.float32)
    bias = np.asarray(bias, dtype=np.float32)

    from concourse.bass_utils import run_bass_kernel_spmd

    if "nc" not in _cache:
        _cache["nc"] = _build_nc()
    nc = _cache["nc"]

    in_maps = _prep_host_inputs(inputs, signatures, forget_kernel,
                                input_kernel, recurrent_kernel, bias)
    res = run_bass_kernel_spmd(nc, in_maps, list(range(NCORES)), trace=_trace)

    out = np.empty((B, U), np.float32)
    for c in range(NCORES):
        h2T = res.results[c]["h_out"]                 # [128, KC*BL] == 2h
        h = 0.5 * h2T.reshape(128, KC, BL).transpose(2, 1, 0).reshape(BL, U)
        out[c * BL:(c + 1) * BL] = h
    if _trace:
        return out, res
    return out


# revision 16
# speedup vs baseline: 1.1520x; 1.1520x over previous
"""Trainium2 Bass kernel for EfmLSTM (signature-gated LSTM), 8-core data-parallel.

Strategy
--------
Data-parallel over batch: B=64 -> 8 cores x B_loc=8. Everything on-chip uses a
"units-on-partition" transposed layout so the sequential scan needs no
transposes at all:

  h^T, c^T, f^T, gate tensors are [128 partitions, u*8+b] where unit = 128*u+p.

Per timestep (per core):
  gates^T: 12 chunks of (gate_type, unit_chunk) x 8 batch =
    sum_k W_rec[k-chunk, m-chunk]-stationary @ h^T[:, k-chunk]  (48 bf16
    matmuls, N=8 moving) accumulated in 3 PSUM banks (one per gate group so
    the c~/i elementwise chains overlap the o matmuls), + x^T_t via DVE,
  then ACT sigmoid/tanh on [128, 32] slices, DVE for the c/h updates.

x^T = inputs @ W_in and f^T = sigmoid(signatures @ W_f + b_f) are precomputed
per 128-step chunk with full-width matmuls (cheap), biases folded in at the
PSUM->SBUF eviction.

The T loop is a hardware For_i over chunk PAIRS (ping-pong SBUF slots inside
the body) — keeps the program ~16K instructions instead of ~60K.

Host side pre-permutes weights into gate order [c~, i, o] and pre-transposes /
pre-casts inputs to bf16, so the device never transposes anything.
"""

import numpy as np
import ml_dtypes

# Problem shapes (hardcoded per harness contract)
B, T, F = 64, 1024, 256
U = 512
SIG = 31
NCORES = 8
BL = B // NCORES  # 8 batch per core

T_CHUNK = 128
KC = U // 128        # 4 k-chunks of h/units
MC = (3 * U) // 128  # 12 gate-column chunks
FC = F // 128        # 2 k-chunks of input features

_cache = {}


DROP_SAME_ENGINE_WAITS = True


def _split_excess_waits(nc, limit=1):
    """Post-pass over sync_info:

    1. Drop semaphore waits on the instruction's OWN engine's sem - engines
       execute their queue in order, so program order already serializes
       same-engine producer/consumer pairs; the sem round-trip (~90ns) is
       pure critical-path cost. (DMA/event sems and cross-engine sems kept.)
    2. This walrus build rejects >1 sync-wait command per instruction
       ('Too many sync wait commands', CoreV2/V3 setupSyncWait). Hoist excess
       waits onto same-engine NoOp instructions inserted just before the
       offending instruction. Keep the LATEST-RESOLVING wait on the
       instruction itself (heuristic: prefer the Activation-sem wait - in the
       scan chain ACT is always the late producer), so the critical wait is
       not stuck behind an extra NoOp decode.
    """
    import concourse.mybir as mybir
    import bass_rust as _br

    own_droppable = {"PE", "Activation", "DVE", "Pool"}
    cnt = 0
    for f in nc.m.functions:
        for b in f.blocks:
            il = b.instructions
            new = []
            for inst in il:
                si = inst.sync_info
                waits = list(si.on_wait) if si and si.on_wait else []
                if not waits:
                    new.append(inst)
                    continue
                eng = str(inst.engine).split(".")[-1]
                if DROP_SAME_ENGINE_WAITS and eng in own_droppable:
                    kept = []
                    for w in waits:
                        nm = (w.ant_name or "") if w.sync_type == "semaphore" else ""
                        pref = nm.rsplit("_", 1)[0] if nm else ""
                        if pref == eng:
                            continue
                        kept.append(w)
                    waits = kept
                if len(waits) > limit:
                    act_i = None
                    for i_, w in enumerate(waits):
                        nm = (w.ant_name or "") if w.sync_type == "semaphore" else ""
                        if nm.startswith("Activation"):
                            act_i = i_
                    if act_i is not None:
                        waits.append(waits.pop(act_i))
                    for w in waits[:-limit]:
                        nop = mybir.InstNoOp(name=f"wsplit_{cnt}", ins=[], outs=[])
                        cnt += 1
                        nop.engine = inst.engine
                        nop.sync_info = _br.SyncInfo(on_wait=[w], on_update=[])
                        new.append(nop)
                    waits = waits[-limit:]
                si.on_wait = waits
                new.append(inst)
            il[:] = new
    return cnt


def _build_nc(compute_dt_name="bfloat16", t_eff=T):
    import concourse.bass as bass
    import concourse.mybir as mybir
    import concourse.tile as tile
    from contextlib import ExitStack

    fp32 = mybir.dt.float32
    cdt = getattr(mybir.dt, compute_dt_name)
    AF = mybir.ActivationFunctionType
    ALU = mybir.AluOpType
    ds = bass.ds

    assert t_eff % (2 * T_CHUNK) == 0
    nc = bass.Bass()

    # t dim padded by one body (2*T_CHUNK) so the loop's next-iteration
    # prefetch never reads out of bounds
    t_pad = t_eff + 2 * T_CHUNK
    x_in = nc.declare_dram_parameter("inputs_t", [F, BL, t_pad], cdt, isOutput=False)
    sig_in = nc.declare_dram_parameter("sig_t", [SIG, BL, t_pad], cdt, isOutput=False)
    wrec_in = nc.declare_dram_parameter("wrec", [128, KC * MC * 128], cdt, isOutput=False)
    win_in = nc.declare_dram_parameter("win", [128, FC * MC * 128], cdt, isOutput=False)
    wsig_in = nc.declare_dram_parameter("wsig", [SIG, U], cdt, isOutput=False)
    bias_g_in = nc.declare_dram_parameter("bias_g", [128, MC], fp32, isOutput=False)
    bias_f_in = nc.declare_dram_parameter("bias_f", [128, KC], fp32, isOutput=False)
    h_out = nc.declare_dram_parameter("h_out", [128, KC * BL], fp32, isOutput=True)

    with ExitStack() as ctx:
        tc = ctx.enter_context(tile.TileContext(nc))

        const = ctx.enter_context(tc.tile_pool(name="const", bufs=1))
        state = ctx.enter_context(tc.tile_pool(name="state", bufs=1))
        data = ctx.enter_context(tc.tile_pool(name="data", bufs=1))
        work = ctx.enter_context(tc.tile_pool(name="work", bufs=3))
        psum_g = ctx.enter_context(tc.tile_pool(name="psum_g", bufs=2, space="PSUM"))
        psum_p = ctx.enter_context(tc.tile_pool(name="psum_p", bufs=2, space="PSUM"))

        wrec = const.tile([128, KC * MC * 128], cdt)
        nc.sync.dma_start(out=wrec[:], in_=wrec_in[:])
        win = const.tile([128, FC * MC * 128], cdt)
        nc.sync.dma_start(out=win[:], in_=win_in[:])
        wsig = const.tile([SIG, U], cdt)
        nc.sync.dma_start(out=wsig[:], in_=wsig_in[:])
        bias_g = const.tile([128, MC], fp32)
        nc.sync.dma_start(out=bias_g[:], in_=bias_g_in[:])
        bias_f = const.tile([128, KC], fp32)
        nc.sync.dma_start(out=bias_f[:], in_=bias_f_in[:])

        h_bf = state.tile([128, KC * BL], cdt)      # h^T bf16, col = 8*k + b
        c_st = state.tile([128, KC * BL], fp32)     # c^T fp32
        nc.vector.memset(h_bf[:], 0.0)
        nc.vector.memset(c_st[:], 0.0)

        # Warm up the 6 scan PSUM bank slots (3 gate groups x 2 bufs) with a
        # dummy start=True matmul each: this sets every element's has_written
        # bit once and we never clear it again. From then on the scan
        # pre-writes x^T into the bank via DVE and the recurrent matmuls
        # accumulate on top with start=False (a DVE write does not clear
        # has_written - only a start=True matmul does).
        for warm in range(2):
            for gi in range(3):
                pg_t = psum_g.tile([128, KC * BL], fp32, tag=f"pg{gi}", name=f"pg{gi}")
                for jj in range(KC):
                    nc.tensor.matmul(
                        pg_t[:, jj * BL:(jj + 1) * BL],
                        lhsT=wrec[:, jj * 128:(jj + 1) * 128],
                        rhs=h_bf[:, 0:BL],
                        start=True, stop=True,
                    )

        srcv = x_in.rearrange("(k p) b t -> p k b t", p=128)

        # per-half static tile sets (ping-pong inside the For_i body)
        halves = []
        for hf in range(2):
            in_sb = data.tile([128, FC * BL * T_CHUNK], cdt, name=f"in_sb{hf}")
            sig_sb = data.tile([SIG, BL * T_CHUNK], cdt, name=f"sig_sb{hf}")
            x_sb = data.tile([128, T_CHUNK * MC * BL], cdt, name=f"x_sb{hf}")
            f_sb = data.tile([128, T_CHUNK * KC * BL], cdt, name=f"f_sb{hf}")
            halves.append((in_sb, sig_sb, x_sb, f_sb))

        NT = 512 // BL  # timesteps covered per 512-wide matmul
        W = KC * BL     # 32

        def emit_dmas(t0sc, hf):
            """Stage the half's inputs+signatures from DRAM."""
            in_sb, sig_sb, x_sb, f_sb = halves[hf]
            in_sbv = in_sb.rearrange("p (k b t) -> p k b t", k=FC, b=BL)
            for k in range(FC):
                nc.sync.dma_start(out=in_sbv[:, k, :, :],
                                  in_=srcv[:, k, :, ds(t0sc, T_CHUNK)])
            nc.sync.dma_start(out=sig_sb.rearrange("p (b t) -> p b t", b=BL),
                              in_=sig_in[:, :, ds(t0sc, T_CHUNK)])

        def pre_groups(hf):
            """Generator: one x/f precompute group (matmuls + eviction) per
            next() — lets the scan interleave these into its PE/ACT idle
            windows."""
            in_sb, sig_sb, x_sb, f_sb = halves[hf]
            in_sb4 = in_sb.rearrange("p (k b t) -> p k b t", k=FC, b=BL)
            x_sb4 = x_sb.rearrange("p (t m b) -> p m b t", m=MC, b=BL)
            f_sb4 = f_sb.rearrange("p (t u b) -> p u b t", u=KC, b=BL)
            sig_sb3 = sig_sb.rearrange("p (b t) -> p b t", b=BL)
            for j in range(MC):
                for th in range(T_CHUNK // NT):
                    ps = psum_p.tile([128, 512], fp32, tag="ps_pre", name="ps_pre")
                    for k in range(FC):
                        nc.tensor.matmul(
                            ps[:],
                            lhsT=win[:, (k * MC + j) * 128:(k * MC + j + 1) * 128],
                            rhs=in_sb4[:, k, :, th * NT:(th + 1) * NT],
                            start=(k == 0), stop=(k == FC - 1),
                        )
                    dst = x_sb4[:, j, :, th * NT:(th + 1) * NT]
                    nc.scalar.activation(
                        dst, ps[:], AF.Identity, bias=bias_g[:, j:j + 1])
                    yield
            for u in range(KC):
                for th in range(T_CHUNK // NT):
                    ps = psum_p.tile([128, 512], fp32, tag="ps_pre", name="ps_pre")
                    nc.tensor.matmul(
                        ps[:],
                        lhsT=wsig[:, u * 128:(u + 1) * 128],
                        rhs=sig_sb3[:, :, th * NT:(th + 1) * NT],
                        start=True, stop=True,
                    )
                    dst = f_sb4[:, u, :, th * NT:(th + 1) * NT]
                    nc.scalar.activation(
                        dst, ps[:], AF.Sigmoid, bias=bias_f[:, u:u + 1])
                    yield

        def scan(hf, pre_iter):
            in_sb, sig_sb, x_sb, f_sb = halves[hf]
            s_o = tc_t = None
            for tt in range(T_CHUNK):
                xs = x_sb[:, tt * MC * BL:(tt + 1) * MC * BL]
                pgs = []
                for gi in range(3):
                    pg_t = psum_g.tile([128, W], fp32, tag=f"pg{gi}", name=f"pg{gi}")
                    # pre-write x^T into the bank; the matmuls accumulate on
                    # top (has_written bits are permanently set, see warmup)
                    nc.vector.tensor_copy(pg_t[:], xs[:, gi * W:(gi + 1) * W])
                    pgs.append(pg_t)
                # c = f*c can start as soon as the prior step's tanh(c) read it
                nc.vector.scalar_tensor_tensor(
                    c_st[:], f_sb[:, tt * W:(tt + 1) * W], 1.0, c_st[:],
                    op0=ALU.mult, op1=ALU.mult)
                # 48 matmuls: m-outer (c~ 0-3, i 4-7, o 8-11), k-inner
                for j in range(MC):
                    gi, jj = j // 4, j % 4
                    for k in range(KC):
                        nc.tensor.matmul(
                            pgs[gi][:, jj * BL:(jj + 1) * BL],
                            lhsT=wrec[:, (k * MC + j) * 128:(k * MC + j + 1) * 128],
                            rhs=h_bf[:, k * BL:(k + 1) * BL],
                            start=False, stop=(k == KC - 1),
                            skip_group_check=True,
                        )
                # activations straight from PSUM; sigma(o) is emitted before
                # tanh(c) so ACT doesn't queue it behind the c chain
                s_cc = work.tile([128, W], fp32, tag="s_cc", name="s_cc")
                nc.scalar.activation(s_cc[:], pgs[0][:], AF.Tanh)
                s_i = work.tile([128, W], fp32, tag="s_i", name="s_i")
                nc.scalar.activation(s_i[:], pgs[1][:], AF.Sigmoid)
                s_o = work.tile([128, W], fp32, tag="s_o", name="s_o")
                nc.scalar.activation(s_o[:], pgs[2][:], AF.Sigmoid)
                tmp = work.tile([128, W], fp32, tag="tmp", name="tmp")
                nc.vector.scalar_tensor_tensor(
                    tmp[:], s_i[:], 1.0, s_cc[:], op0=ALU.mult, op1=ALU.mult)
                nc.vector.scalar_tensor_tensor(
                    c_st[:], c_st[:], 1.0, tmp[:], op0=ALU.mult, op1=ALU.add)
                tc_t = work.tile([128, W], fp32, tag="tc", name="tc")
                nc.scalar.activation(tc_t[:], c_st[:], AF.Tanh)
                nc.vector.scalar_tensor_tensor(
                    h_bf[:], s_o[:], 1.0, tc_t[:], op0=ALU.mult, op1=ALU.mult)
                # fill the PE's h-wait idle window with precompute matmuls
                if pre_iter is not None and tt % 4 == 3:
                    next(pre_iter, None)
            if pre_iter is not None:
                for _ in pre_iter:
                    pass
            return s_o, tc_t

        # prologue: half0 of the first iteration, serial
        emit_dmas(0, 0)
        for _ in pre_groups(0):
            pass

        with tc.For_i(0, t_eff, 2 * T_CHUNK) as t0:
            emit_dmas(t0 + T_CHUNK, 1)           # this iteration's half1 data
            s_o0, tc0 = scan(0, pre_groups(1))   # scan half0, prep half1
            emit_dmas(t0 + 2 * T_CHUNK, 0)       # next iteration's half0 data
            s_o1, tc1 = scan(1, pre_groups(0))   # scan half1, prep next half0

        # final h in fp32 from the last step's stashed (static-slot) tiles
        h_f = state.tile([128, KC * BL], fp32)
        nc.vector.scalar_tensor_tensor(
            h_f[:], s_o1[:], 1.0, tc1[:], op0=ALU.mult, op1=ALU.mult)
        nc.sync.dma_start(out=h_out[:], in_=h_f[:])

    _split_excess_waits(nc)
    return nc


def _prep_host_inputs(inputs, signatures, forget_kernel, input_kernel,
                      recurrent_kernel, bias, cdt=ml_dtypes.bfloat16, t_factor=1):
    """Host-side shard + permute + transpose + cast. Returns in_maps list."""
    # gate order in reference: [i, c~, o]; ours: [c~, i, o]
    perm = np.concatenate([np.arange(U, 2 * U), np.arange(0, U), np.arange(2 * U, 3 * U)])
    win_p = input_kernel[:, perm]          # [F, 3U]
    wrec_p = recurrent_kernel[:, perm]     # [U, 3U]
    b_i, b_f, b_c, b_o = np.split(bias, 4)
    bias_g = np.concatenate([b_c, b_i, b_o])  # per permuted gate col, [3U]

    # wrec blocks: [128, (k*MC + j)*128 + c] = wrec_p[128*k + p, 128*j + c]
    wr = wrec_p.reshape(KC, 128, MC, 128).transpose(1, 0, 2, 3).reshape(128, KC * MC * 128)
    wi = win_p.reshape(FC, 128, MC, 128).transpose(1, 0, 2, 3).reshape(128, FC * MC * 128)
    bg = bias_g.reshape(MC, 128).T.copy()          # [128, MC]
    bf_ = b_f.reshape(KC, 128).T.copy()            # [128, KC]

    wr = wr.astype(cdt)
    wi = wi.astype(cdt)
    wsig = forget_kernel.astype(cdt)               # [SIG, U]

    in_maps = []
    for c in range(NCORES):
        bsl = slice(c * BL, (c + 1) * BL)
        # [BL, T, F] -> [F, BL, T]
        x_t = np.ascontiguousarray(inputs[bsl].transpose(2, 0, 1)).astype(cdt)
        s_t = np.ascontiguousarray(signatures[bsl].transpose(2, 0, 1)).astype(cdt)
        if t_factor > 1:
            x_t = np.tile(x_t, (1, 1, t_factor))
            s_t = np.tile(s_t, (1, 1, t_factor))
        pad = 2 * T_CHUNK
        x_t = np.concatenate([x_t, np.zeros(x_t.shape[:2] + (pad,), x_t.dtype)], axis=2)
        s_t = np.concatenate([s_t, np.zeros(s_t.shape[:2] + (pad,), s_t.dtype)], axis=2)
        in_maps.append({
            "inputs_t": x_t, "sig_t": s_t, "wrec": wr, "win": wi,
            "wsig": wsig, "bias_g": bg.astype(np.float32),
            "bias_f": bf_.astype(np.float32),
        })
    return in_maps


def kernel(inputs, signatures, forget_kernel, input_kernel, recurrent_kernel,
           bias, _trace=False):
    inputs = np.asarray(inputs, dtype=np.float32)
    signatures = np.asarray(signatures, dtype=np.float32)
    forget_kernel = np.asarray(forget_kernel, dtype=np.float32)
    input_kernel = np.asarray(input_kernel, dtype=np.float32)
    recurrent_kernel = np.asarray(recurrent_kernel, dtype=np.float32)
    bias = np.asarray(bias, dtype=np.float32)

    from concourse.bass_utils import run_bass_kernel_spmd

    if "nc" not in _cache:
        _cache["nc"] = _build_nc()
    nc = _cache["nc"]

    in_maps = _prep_host_inputs(inputs, signatures, forget_kernel,
                                input_kernel, recurrent_kernel, bias)
    res = run_bass_kernel_spmd(nc, in_maps, list(range(NCORES)), trace=_trace)

    out = np.empty((B, U), np.float32)
    for c in range(NCORES):
        hT = res.results[c]["h_out"]                  # [128, KC*BL]
        h = hT.reshape(128, KC, BL).transpose(2, 1, 0).reshape(BL, U)
        out[c * BL:(c + 1) * BL] = h
    if _trace:
        return out, res
    return out

